# revision 1
# baseline (speedup 1.0000x reference)
"""Multi-head attention Trainium2 Bass kernel.

Shapes (hardcoded): B=4, T=2048, E=1024, H=16, DK=64.
Sharding over 8 cores: core c -> (batch b = c//2, head-group g = c%2).
Each core computes 8 heads of one batch end-to-end and a partial output
projection; the host sums the two partials per batch.

Layout strategy (everything transposed so no on-device transposes):
  - inputs fed as x^T [E, T] (host-transposed)
  - Q^T, K^T kept as [f_local, T] (f on partitions)
  - V kept natural [T, f_local], stored per-head with an appended
    ones-column so attn@V also produces softmax row-sums in PSUM row 64
  - S^T [keys, q] per (head, key-tile); exp fused with 1/sqrt(dk) scale and
    additive mask bias (per-partition) on the scalar engine
  - output projection consumes x^T_local directly as lhsT
"""

import numpy as np

import concourse.bass as bass
import concourse.tile as tile
from concourse import bacc, mybir
from concourse.bass_utils import run_bass_kernel_spmd

F32 = mybir.dt.float32
F32R = mybir.dt.float32r

B, T, E, H = 4, 2048, 1024, 16
DK = E // H            # 64
N_CORES = 8
FL = 512               # local f (8 heads * 64)
HL = 8                 # heads per core
NT = T // 128          # 16 t-tiles
NE = E // 128          # 8 e-tiles
NFT = FL // 128        # 4 local f-tiles
NC4 = T // 512         # 4 t-chunks of 512

BF16 = mybir.dt.bfloat16
DT = BF16


def build_nc():
    nc = bacc.Bacc("TRN2", target_bir_lowering=False, debug=False,
                   enable_asserts=False)

    qT = nc.dram_tensor("qT", [E, T], DT, kind="ExternalInput").ap()
    kT = nc.dram_tensor("kT", [E, T], DT, kind="ExternalInput").ap()
    vT = nc.dram_tensor("vT", [E, T], DT, kind="ExternalInput").ap()
    wqT = nc.dram_tensor("wqT", [E, FL], DT, kind="ExternalInput").ap()
    wkT = nc.dram_tensor("wkT", [E, FL], DT, kind="ExternalInput").ap()
    wvT = nc.dram_tensor("wvT", [E, FL], DT, kind="ExternalInput").ap()
    woT = nc.dram_tensor("woT", [FL, E], DT, kind="ExternalInput").ap()
    bq = nc.dram_tensor("bq", [128, NFT], F32, kind="ExternalInput").ap()
    bk = nc.dram_tensor("bk", [128, NFT], F32, kind="ExternalInput").ap()
    bv = nc.dram_tensor("bv", [1, FL], DT, kind="ExternalInput").ap()
    bo = nc.dram_tensor("bo", [1, E], DT, kind="ExternalInput").ap()
    maskb = nc.dram_tensor("maskb", [128, NT], F32, kind="ExternalInput").ap()
    maskb2 = nc.dram_tensor("maskb2", [128, NT], F32,
                            kind="ExternalInput").ap()
    ones_d = nc.dram_tensor("ones_d", [1, 128], DT, kind="ExternalInput").ap()
    vones = nc.dram_tensor("vones", [128, HL], DT, kind="ExternalInput").ap()
    out = nc.dram_tensor("out", [T, E], F32, kind="ExternalOutput").ap()

    with tile.TileContext(nc) as tc:
        with (
            tc.tile_pool(name="const", bufs=1) as constp,
            tc.tile_pool(name="qkt", bufs=1) as qktp,
            tc.tile_pool(name="vsb", bufs=1) as vsbp,
            tc.tile_pool(name="xtl", bufs=1) as xtlp,
            tc.tile_pool(name="ps_s", bufs=2, space="PSUM") as ps_s,
            tc.tile_pool(name="ps_o", bufs=1, space="PSUM") as ps_o,
            tc.tile_pool(name="ps_w", bufs=1, space="PSUM") as ps_w,
        ):
            # ---- constants ----
            bq_sb = constp.tile([128, NFT], F32, tag="bq")
            nc.sync.dma_start(out=bq_sb[:], in_=bq)
            bk_sb = constp.tile([128, NFT], F32, tag="bk")
            nc.sync.dma_start(out=bk_sb[:], in_=bk)
            bv_sb = constp.tile([1, FL], DT, tag="bv")
            nc.sync.dma_start(out=bv_sb[:], in_=bv)
            bo_sb = constp.tile([1, E], DT, tag="bo")
            nc.sync.dma_start(out=bo_sb[:], in_=bo)
            mask_sb = constp.tile([128, NT], F32, tag="maskb")
            nc.sync.dma_start(out=mask_sb[:], in_=maskb)
            mask2_sb = constp.tile([128, NT], F32, tag="maskb2")
            nc.sync.dma_start(out=mask2_sb[:], in_=maskb2)
            ones_sb = constp.tile([1, 128], DT, tag="ones")
            nc.sync.dma_start(out=ones_sb[:], in_=ones_d)

            # persistent activation storage: per-head tiles, rows 0..63 =
            # head data, rows 64..127 = zeros (pad matmuls to full 128
            # contraction so the PE activity monitor keeps the clock warm)
            qt = [qktp.tile([128, T], DT, tag=f"qt{i}", name=f"qt{i}")
                  for i in range(HL)]
            kt = [qktp.tile([128, T], DT, tag=f"kt{i}", name=f"kt{i}")
                  for i in range(HL)]
            for i in range(HL):
                nc.scalar.memzero(qt[i][64:128, :])
                nc.scalar.memzero(kt[i][64:128, :])
            # V per t-tile: [128, 8 heads * 128]; per head: cols 0..63 = V,
            # col 64 = 1.0 (row-sum trick), cols 65..127 = zeros (padding)
            vt = [vsbp.tile([128, HL * 128], DT, tag=f"v{j}", name=f"v{j}")
                  for j in range(NT)]
            for j in range(NT):
                nc.scalar.memzero(vt[j][:])
            xtl = [xtlp.tile([128, T], DT, tag=f"x{i}", name=f"x{i}")
                   for i in range(NFT)]

            # ---- phase 1a: V projection (natural layout + ones col) ----
            with tc.tile_pool(name="wv", bufs=1) as wvp, \
                 tc.tile_pool(name="vload", bufs=1) as vlp:
                wv_sb = [wvp.tile([128, FL], DT, tag=f"wv{e}", name=f"wv{e}")
                         for e in range(NE)]
                for e in range(NE):
                    nc.sync.dma_start(out=wv_sb[e][:],
                                      in_=wvT[e * 128:(e + 1) * 128, :])
                for hf in range(2):
                    vf = [vlp.tile([128, 1024], DT, tag=f"vf{e}",
                                   name=f"vf{e}") for e in range(NE)]
                    for e in range(NE):
                        nc.sync.dma_start(
                            out=vf[e][:],
                            in_=vT[e * 128:(e + 1) * 128,
                                   hf * 1024:(hf + 1) * 1024])
                    for jj in range(NT // 2):
                        j = hf * (NT // 2) + jj
                        ps = ps_s.tile([128, 1024], F32, tag="ps_s",
                                       name="psv")
                        for e in range(NE):
                            nc.tensor.matmul(
                                ps[:, 0:FL],
                                lhsT=vf[e][:, jj * 128:(jj + 1) * 128],
                                rhs=wv_sb[e][:],
                                start=(e == 0), stop=False)
                        nc.tensor.matmul(ps[:, 0:FL], lhsT=ones_sb[:],
                                         rhs=bv_sb[:], start=False, stop=True)
                        nc.sync.dma_start(
                            out=vt[j].rearrange(
                                "p (h w) -> p h w", w=128)[:, :, 64:65],
                            in_=vones.rearrange("p (h o) -> p h o", o=1))
                        nc.vector.tensor_copy(
                            out=vt[j].rearrange(
                                "p (h w) -> p h w", w=128)[:, :, 0:64],
                            in_=ps[:, 0:FL].rearrange(
                                "p (h w) -> p h w", w=64))

            # ---- phases 1b/2 interleaved: K^T/Q^T projections in two
            # f-passes; attention for heads 0-3 is emitted between the
            # passes so ACT exp work overlaps the remaining projection
            # matmuls on PE ----
            wp = tc.alloc_tile_pool(name="wqk", bufs=1)
            xlp = tc.alloc_tile_pool(name="xload", bufs=9)
            expp = tc.alloc_tile_pool(name="exps", bufs=4)
            normp = tc.alloc_tile_pool(name="norm", bufs=2)

            w_sb = {}
            for name, wdram in (("k", wkT), ("q", wqT)):
                w_sb[name] = [
                    wp.tile([128, FL], DT, tag=f"w{name}{e}",
                            name=f"w{name}{e}") for e in range(NE)]
                for e in range(NE):
                    nc.sync.dma_start(
                        out=w_sb[name][e][:],
                        in_=wdram[e * 128:(e + 1) * 128, :])

            def proj_pass(fl, name, demote=0):
                xdram = kT if name == "k" else qT
                bias_sb = bk_sb if name == "k" else bq_sb
                dst = kt if name == "k" else qt
                for c in range(NC4):
                    xs = []
                    for e in range(NE):
                        xe = xlp.tile([128, 512], DT, tag="xchunk",
                                      name="xchunk")
                        nc.sync.dma_start(
                            out=xe[:],
                            in_=xdram[e * 128:(e + 1) * 128,
                                      c * 512:(c + 1) * 512])
                        xs.append(xe)
                    save = tc.cur_priority
                    if demote:
                        tc.cur_priority = save + demote
                    ps = ps_w.tile([128, 512 * len(fl)], F32, tag="psqk",
                                   name="psqk")
                    for fi, f in enumerate(fl):
                        for e in range(NE):
                            nc.tensor.matmul(
                                ps[:, fi * 512:(fi + 1) * 512],
                                lhsT=w_sb[name][e][:, f * 128:(f + 1) * 128],
                                rhs=xs[e][:],
                                start=(e == 0), stop=(e == NE - 1))
                    for fi, f in enumerate(fl):
                        for hh in range(2):
                            nc.vector.tensor_scalar_add(
                                dst[2 * f + hh][0:64,
                                                c * 512:(c + 1) * 512],
                                ps[hh * 64:(hh + 1) * 64,
                                   fi * 512:(fi + 1) * 512],
                                bias_sb[hh * 64:(hh + 1) * 64,
                                        f:f + 1])
                    if demote:
                        tc.cur_priority = save

            def attention_half(h, half):
                qh = qt[h]
                kh = kt[h]
                if True:
                    qsl = slice(half * 1024, (half + 1) * 1024)
                    pso = ps_o.tile([128, 1024], F32, tag="ps_o",
                                    name="pso")
                    for k in range(NT):
                        pss = ps_s.tile([128, 1024], F32, tag="ps_s",
                                        name="pss")
                        for j in range(2):
                            cj = half * 1024 + j * 512
                            nc.tensor.matmul(
                                pss[:, j * 512:(j + 1) * 512],
                                lhsT=kh[:, k * 128:(k + 1) * 128],
                                rhs=qh[:, cj:cj + 512],
                                start=True, stop=True)
                        es = expp.tile([128, 1024], DT, tag="es",
                                       name="es")
                        nc.scalar.activation(
                            out=es[:], in_=pss[:],
                            func=mybir.ActivationFunctionType.Exp,
                            bias=mask_sb[:, k:k + 1], scale=0.125)
                        for j in range(2):
                            nc.tensor.matmul(
                                pso[:, j * 512:(j + 1) * 512],
                                lhsT=vt[k][:, h * 128:(h + 1) * 128],
                                rhs=es[:, j * 512:(j + 1) * 512],
                                start=(k == 0), stop=(k == NT - 1))
                    # normalize: rows 0..63 = O^T, row 64 = sum(exp)
                    ot = normp.tile([65, 1024], F32, tag="ot", name="ot")
                    nc.vector.tensor_copy(out=ot[:], in_=pso[0:65, :])
                    rep = normp.tile([64, 1024], F32, tag="rep",
                                     name="rep")
                    ri = normp.tile([1, 1024], F32, tag="ri", name="ri")
                    nc.vector.reciprocal(ri[:], ot[64:65, :])
                    nc.sync.dma_start(out=rep[0:1, :], in_=ri[:])
                    for d in range(6):  # 1 -> 64 partitions
                        w = 1 << d
                        nc.sync.dma_start(out=rep[w:2 * w, :],
                                          in_=rep[0:w, :])
                    nc.vector.tensor_mul(
                        xtl[h // 2][h % 2 * 64:h % 2 * 64 + 64, qsl],
                        ot[0:64, :], rep[:])

            proj_pass([0], "k")
            proj_pass([0], "q")

            # ---- phase 3: output projection (partial) ----
            wop = tc.alloc_tile_pool(name="wo", bufs=1)
            osbp = tc.alloc_tile_pool(name="osb", bufs=2)
            wo_sb = [wop.tile([128, E], DT, tag=f"wo{e}", name=f"wo{e}")
                     for e in range(NFT)]
            for e in range(NFT):
                nc.sync.dma_start(out=wo_sb[e][:],
                                  in_=woT[e * 128:(e + 1) * 128, :])

            def final_proj(js):
                for j in js:
                    ps = ps_w.tile([128, E], F32, tag="psqk", name="psf")
                    for c2 in range(2):
                        nc.tensor.matmul(
                            ps[:, c2 * 512:(c2 + 1) * 512],
                            lhsT=ones_sb[:],
                            rhs=bo_sb[:, c2 * 512:(c2 + 1) * 512],
                            start=True, stop=False)
                    for e in range(NFT):
                        for c2 in range(2):
                            nc.tensor.matmul(
                                ps[:, c2 * 512:(c2 + 1) * 512],
                                lhsT=xtl[e][:, j * 128:(j + 1) * 128],
                                rhs=wo_sb[e][:, c2 * 512:(c2 + 1) * 512],
                                start=False, stop=(e == NFT - 1))
                    ob = osbp.tile([128, E], F32, tag="ob", name="ob")
                    nc.vector.tensor_copy(out=ob[:], in_=ps[:])
                    nc.sync.dma_start(out=out[j * 128:(j + 1) * 128, :],
                                      in_=ob[:])

            # phase A: all heads, first q-half; then final rows 0..1023
            attention_half(0, 0)
            proj_pass([1], "k")
            attention_half(1, 0)
            proj_pass([1], "q")
            attention_half(2, 0)
            proj_pass([2, 3], "k")
            attention_half(3, 0)
            proj_pass([2, 3], "q")
            for h in range(4, HL):
                attention_half(h, 0)
            final_proj(range(NT // 2))
            # phase B: second q-half; then final rows 1024..2047
            for h in range(HL):
                attention_half(h, 1)
            final_proj(range(NT // 2, NT))
            for p in (osbp, wop, normp, expp, xlp, wp):
                p.release()

    nc.compile()
    return nc


_NC_CACHE = None


def _get_nc():
    global _NC_CACHE
    if _NC_CACHE is None:
        _NC_CACHE = build_nc()
    return _NC_CACHE


def make_in_maps(query, key_, value, mask, w_q, b_q, w_k, b_k, w_v, b_v,
                 w_o, b_o):
    import ml_dtypes
    f32 = np.float32
    bf16 = ml_dtypes.bfloat16
    c = lambda a: np.ascontiguousarray(a).astype(bf16)
    in_maps = []
    for core in range(N_CORES):
        b, g = core // 2, core % 2
        fs = slice(g * FL, (g + 1) * FL)
        mb = np.where(mask[b], 0.0, -30.0).astype(f32)
        in_maps.append({
            "qT": c(query[b].T.astype(f32, copy=False)),
            "kT": c(key_[b].T.astype(f32, copy=False)),
            "vT": c(value[b].T.astype(f32, copy=False)),
            "wqT": c(w_q[fs, :].T.astype(f32, copy=False)),
            "wkT": c(w_k[fs, :].T.astype(f32, copy=False)),
            "wvT": c(w_v[fs, :].T.astype(f32, copy=False)),
            "woT": c(w_o[:, fs].T.astype(f32, copy=False)),
            "bq": np.ascontiguousarray(
                b_q[fs].astype(f32, copy=False).reshape(NFT, 128).T),
            "bk": np.ascontiguousarray(
                b_k[fs].astype(f32, copy=False).reshape(NFT, 128).T),
            "bv": b_v[fs].reshape(1, FL).astype(bf16),
            "bo": (b_o.astype(f32, copy=False) if g == 0
                   else np.zeros(E, f32)).reshape(1, E).astype(bf16),
            "maskb": np.ascontiguousarray(mb.reshape(NT, 128).T),
            "maskb2": np.ascontiguousarray(
                (mb * 12102203.161561485 + 1064986823.0).astype(f32)
                .reshape(NT, 128).T),
            "ones_d": np.ones((1, 128), bf16),
            "vones": np.ones((128, HL), bf16),
        })
    return in_maps


def kernel(query=None, key_=None, value=None, mask=None, w_q=None, b_q=None,
           w_k=None, b_k=None, w_v=None, b_v=None, w_o=None, b_o=None,
           key=None, **_kwargs):
    if key_ is None:
        key_ = key
    args = [np.asarray(a) for a in
            (query, key_, value, mask, w_q, b_q, w_k, b_k, w_v, b_v,
             w_o, b_o)]
    nc = _get_nc()
    in_maps = make_in_maps(*args)
    res = run_bass_kernel_spmd(nc, in_maps, core_ids=list(range(N_CORES)))
    outs = [res.results[i]["out"] for i in range(N_CORES)]
    full = np.empty((B, T, E), np.float32)
    for b in range(B):
        full[b] = outs[2 * b] + outs[2 * b + 1]
    return full



# revision 11
# speedup vs baseline: 1.3673x; 1.3673x over previous
"""Multi-head attention Trainium2 Bass kernel.

Shapes (hardcoded): B=4, T=2048, E=1024, H=16, DK=64.
Sharding over 8 cores: core c -> (batch b = c//2, head-group g = c%2).
Each core computes 8 heads of one batch end-to-end and a partial output
projection; the host sums the two partials per batch.

Layout strategy (everything transposed so no on-device transposes):
  - inputs fed as x^T [E, T] (host-transposed)
  - Q^T, K^T kept as [65, T] per head (rows 0..63 = head dims; row 64 =
    ones for Q / scaled mask bias for K, so the scores matmul computes
    s + maskbias in one 65-deep contraction)
  - V kept natural [T, f_local], stored per-head 72 cols wide with an
    appended ones-column so attn@V also produces softmax row-sums
  - S^T [keys, q] per (head, k-tile-pair, q-chunk); exp fused with
    1/sqrt(dk) scale on the scalar engine
  - normalize via reciprocal_approx_fast + gpsimd partition_broadcast
  - output projection consumes x^T_local directly as lhsT

Engine assignment: PE = matmuls, ACT = exp only, DVE = recip +
normalize-mul + O-proj psum drains, Pool = all other copies/bias adds.
"""

import numpy as np

import concourse.bass as bass
import concourse.tile as tile
from concourse import bacc, mybir
from concourse.bass_utils import run_bass_kernel_spmd

F32 = mybir.dt.float32

B, T, E, H = 4, 2048, 1024, 16
DK = E // H            # 64
N_CORES = 8
FL = 512               # local f (8 heads * 64)
HL = 8                 # heads per core
NT = T // 128          # 16 t-tiles
NE = E // 128          # 8 e-tiles
NFT = FL // 128        # 4 local f-tiles (head pairs)
VW = 72                # per-head width in vt tiles (64 data + 1 ones + pad)

BF16 = mybir.dt.bfloat16
DT = BF16
Exp = mybir.ActivationFunctionType.Exp
MUL = mybir.AluOpType.mult


def build_nc():
    nc = bacc.Bacc("TRN2", target_bir_lowering=False, debug=False,
                   enable_asserts=False)

    qT = nc.dram_tensor("qT", [E, T], DT, kind="ExternalInput").ap()
    kT = nc.dram_tensor("kT", [E, T], DT, kind="ExternalInput").ap()
    vT = nc.dram_tensor("vT", [E, T], DT, kind="ExternalInput").ap()
    wqT = nc.dram_tensor("wqT", [E, FL], DT, kind="ExternalInput").ap()
    wkT = nc.dram_tensor("wkT", [E, FL], DT, kind="ExternalInput").ap()
    wvT = nc.dram_tensor("wvT", [E, FL], DT, kind="ExternalInput").ap()
    woT = nc.dram_tensor("woT", [FL, E], DT, kind="ExternalInput").ap()
    bq = nc.dram_tensor("bq", [128, NFT], F32, kind="ExternalInput").ap()
    bk = nc.dram_tensor("bk", [128, NFT], F32, kind="ExternalInput").ap()
    bv = nc.dram_tensor("bv", [1, FL], DT, kind="ExternalInput").ap()
    bo = nc.dram_tensor("bo", [1, E], DT, kind="ExternalInput").ap()
    ones_d = nc.dram_tensor("ones_d", [1, 128], DT, kind="ExternalInput").ap()
    onesrow = nc.dram_tensor("onesrow", [1, T], DT,
                             kind="ExternalInput").ap()
    maskrow = nc.dram_tensor("maskrow", [1, T], DT,
                             kind="ExternalInput").ap()
    out = nc.dram_tensor("out", [T, E], F32, kind="ExternalOutput").ap()

    with tile.TileContext(nc) as tc:
        with (
            tc.tile_pool(name="const", bufs=1) as constp,
            tc.tile_pool(name="qkt", bufs=1) as qktp,
            tc.tile_pool(name="vsb", bufs=1) as vsbp,
            tc.tile_pool(name="xtl", bufs=1) as xtlp,
            tc.tile_pool(name="wqk", bufs=1) as wp,
            tc.tile_pool(name="wo", bufs=1) as wop,
            tc.tile_pool(name="xload", bufs=12) as xlp,
            tc.tile_pool(name="vload", bufs=8) as vlp,
            tc.tile_pool(name="exps", bufs=4) as expp,
            tc.tile_pool(name="norm", bufs=2) as normp,
            tc.tile_pool(name="osb", bufs=2) as osbp,
            tc.tile_pool(name="ps_s", bufs=2, space="PSUM") as ps_s,
            tc.tile_pool(name="ps_o", bufs=2, space="PSUM") as ps_o,
            tc.tile_pool(name="ps_w", bufs=2, space="PSUM") as ps_w,
        ):
            # ---- constants ----
            bq_sb = constp.tile([128, NFT], F32, tag="bq")
            nc.sync.dma_start(out=bq_sb[:], in_=bq)
            bk_sb = constp.tile([128, NFT], F32, tag="bk")
            nc.sync.dma_start(out=bk_sb[:], in_=bk)
            bv_sb = constp.tile([1, FL], DT, tag="bv")
            nc.sync.dma_start(out=bv_sb[:], in_=bv)
            bo_sb = constp.tile([1, E], DT, tag="bo")
            nc.sync.dma_start(out=bo_sb[:], in_=bo)
            ones_sb = constp.tile([1, 128], DT, tag="ones")
            nc.sync.dma_start(out=ones_sb[:], in_=ones_d)

            # persistent per-head Q^T/K^T tiles: rows 0..63 = head data,
            # row 64 = ones (Q) / scaled mask bias (K)
            qt = [qktp.tile([65, T], DT, tag=f"qt{i}", name=f"qt{i}")
                  for i in range(HL)]
            kt = [qktp.tile([65, T], DT, tag=f"kt{i}", name=f"kt{i}")
                  for i in range(HL)]
            for i in range(HL):
                nc.sync.dma_start(out=qt[i][64:65, :], in_=onesrow)
                nc.sync.dma_start(out=kt[i][64:65, :], in_=maskrow)
            # V per t-tile: [128, 8 heads * 72]; per head: cols 0..63 = V,
            # col 64 = 1.0 (row-sum trick), cols 65..71 unused
            vt = [vsbp.tile([128, HL * VW], DT, tag=f"v{j}", name=f"v{j}")
                  for j in range(NT)]
            xtl = [xtlp.tile([128, T], DT, tag=f"x{i}", name=f"x{i}")
                   for i in range(NFT)]

            # ---- weights ----
            w_sb = {}
            for name, wdram in (("k", wkT), ("q", wqT)):
                w_sb[name] = [
                    wp.tile([128, FL], DT, tag=f"w{name}{e}",
                            name=f"w{name}{e}") for e in range(NE)]
                for e in range(NE):
                    nc.sync.dma_start(
                        out=w_sb[name][e][:],
                        in_=wdram[e * 128:(e + 1) * 128, :])
            wv_sb = [wp.tile([128, FL], DT, tag=f"wv{e}", name=f"wv{e}")
                     for e in range(NE)]
            for e in range(NE):
                nc.sync.dma_start(out=wv_sb[e][:],
                                  in_=wvT[e * 128:(e + 1) * 128, :])
            wo_sb = [wop.tile([128, E], DT, tag=f"wo{e}", name=f"wo{e}")
                     for e in range(NFT)]
            for e in range(NFT):
                nc.sync.dma_start(out=wo_sb[e][:],
                                  in_=woT[e * 128:(e + 1) * 128, :])

            # ---------- emission helpers (generators yield between PE
            # chunks so projection work interleaves into attention) ----

            def kq_proj_units(fs):
                """K/Q projection for f-tiles (head-pairs) in fs,
                pair-major so earlier pairs complete first (x chunks are
                re-streamed per pair; the extra HBM traffic hides).
                One yield-unit = one (f, chunk, name) psum tile."""
                for f in fs:
                    for c in range(4):
                        xs = {}
                        for nm, xdram in (("k", kT), ("q", qT)):
                            xs[nm] = []
                            for e in range(NE):
                                xe = xlp.tile([128, 512], DT, tag="xchunk",
                                              name="xchunk")
                                nc.sync.dma_start(
                                    out=xe[:],
                                    in_=xdram[e * 128:(e + 1) * 128,
                                              c * 512:(c + 1) * 512])
                                xs[nm].append(xe)
                        for nm in ("k", "q"):
                            bias_sb = bk_sb if nm == "k" else bq_sb
                            dsts = kt if nm == "k" else qt
                            ps = ps_w.tile([128, 512], F32, tag="psw",
                                           name="psw")
                            for e in range(NE):
                                nc.tensor.matmul(
                                    ps[:],
                                    lhsT=w_sb[nm][e][:, f * 128:(f + 1) * 128],
                                    rhs=xs[nm][e][:],
                                    start=(e == 0), stop=(e == NE - 1))
                            for hh in range(2):
                                nc.vector.tensor_scalar_add(
                                    dsts[2 * f + hh][0:64,
                                                     c * 512:(c + 1) * 512],
                                    ps[hh * 64:(hh + 1) * 64, :],
                                    bias_sb[hh * 64:(hh + 1) * 64, f:f + 1])
                            yield

            def v_proj_units():
                """V projection. One yield-unit = one t-tile."""
                for hf in range(2):
                    vf = [vlp.tile([128, 1024], DT, tag="vf", name="vf")
                          for _ in range(NE)]
                    for e in range(NE):
                        nc.sync.dma_start(
                            out=vf[e][:],
                            in_=vT[e * 128:(e + 1) * 128,
                                   hf * 1024:(hf + 1) * 1024])
                    for jj in range(NT // 2):
                        j = hf * (NT // 2) + jj
                        ps = ps_w.tile([128, 512], F32, tag="psw",
                                       name="psv")
                        for e in range(NE):
                            nc.tensor.matmul(
                                ps[:],
                                lhsT=vf[e][:, jj * 128:(jj + 1) * 128],
                                rhs=wv_sb[e][:],
                                start=(e == 0), stop=False)
                        nc.tensor.matmul(ps[:], lhsT=ones_sb[:],
                                         rhs=bv_sb[:], start=False, stop=True)
                        nc.vector.tensor_copy(
                            out=vt[j].rearrange(
                                "p (h w) -> p h w", w=VW)[:, :, 0:64],
                            in_=ps[:].rearrange(
                                "p (h w) -> p h w", w=64))
                        nc.gpsimd.memset(
                            vt[j].rearrange(
                                "p (h w) -> p h w", w=VW)[:, :, 64:65],
                            1.0)
                        yield

            def o_proj_units(js):
                """Output projection. One yield-unit = one (j, c2)."""
                for j in js:
                    ob = osbp.tile([128, E], F32, tag="ob", name="ob")
                    for c2 in range(2):
                        ps = ps_w.tile([128, 512], F32, tag="psw",
                                       name="psf")
                        nc.tensor.matmul(
                            ps[:], lhsT=ones_sb[:],
                            rhs=bo_sb[:, c2 * 512:(c2 + 1) * 512],
                            start=True, stop=False)
                        for e in range(NFT):
                            nc.tensor.matmul(
                                ps[:],
                                lhsT=xtl[e][:, j * 128:(j + 1) * 128],
                                rhs=wo_sb[e][:, c2 * 512:(c2 + 1) * 512],
                                start=False, stop=(e == NFT - 1))
                        nc.vector.tensor_copy(
                            out=ob[:, c2 * 512:(c2 + 1) * 512], in_=ps[:])
                        yield
                    nc.sync.dma_start(out=out[j * 128:(j + 1) * 128, :],
                                      in_=ob[:])

            # ---- attention: software-pipelined scores/exp/av stream ----
            # one unit u = (head, half, chunk, kpair): 2 score matmuls
            # [128,512] into one [128,1024] psum tile, 1 exp, 2 av matmuls.

            def att_units(h, half):
                """Yields (emit_scores, emit_exp_av) closures per unit."""
                qh = qt[h]
                kh = kt[h]
                for c in range(2):
                    q0 = half * 1024 + c * 512
                    pso = ps_o.tile([65, 512], F32, tag="pso", name="pso")
                    for kp in range(NT // 2):
                        def mk(kp=kp, c=c, q0=q0, pso=pso, qh=qh, kh=kh,
                               h=h, half=half):
                            st = {}

                            def scores():
                                ps = ps_s.tile([128, 1024], F32, tag="ps_s",
                                               name="pss")
                                for i in range(2):
                                    k = 2 * kp + i
                                    nc.tensor.matmul(
                                        ps[:, i * 512:(i + 1) * 512],
                                        lhsT=kh[:, k * 128:(k + 1) * 128],
                                        rhs=qh[:, q0:q0 + 512],
                                        start=True, stop=True)
                                st["ps"] = ps

                            def exp_av():
                                es = expp.tile([128, 1024], DT, tag="es",
                                               name="es")
                                nc.scalar.activation(
                                    out=es[:], in_=st["ps"][:], func=Exp,
                                    scale=0.125)
                                for i in range(2):
                                    k = 2 * kp + i
                                    nc.tensor.matmul(
                                        pso[:],
                                        lhsT=vt[k][:, h * VW:h * VW + 65],
                                        rhs=es[:, i * 512:(i + 1) * 512],
                                        start=(kp == 0 and i == 0),
                                        stop=(kp == NT // 2 - 1 and i == 1))
                            return scores, exp_av
                        yield mk() + (pso, c)

            def normalize(h, half, oT, zraw):
                """After both chunks of (h, half) staged into oT/zraw.
                zraw is a partition-0 [1,1024] tile — custom-DVE recip
                breaks on partition-offset inputs, so the row-sum row is
                staged separately instead of read from oT[64:65]."""
                rep = normp.tile([64, 1024], F32, tag="rep", name="rep")
                nc.vector.reciprocal_approx_fast(rep[0:1, :], zraw[:])
                nc.gpsimd.partition_broadcast(rep[:], rep[0:1, :])
                f = h // 2
                r0 = (h % 2) * 64
                qsl = slice(half * 1024, (half + 1) * 1024)
                nc.vector.tensor_tensor(
                    out=xtl[f][r0:r0 + 64, qsl],
                    in0=oT[:], in1=rep[:], op=MUL)

            def attention_half(half, fillers):
                """Run all 8 heads' attention for one q-half, pulling
                filler PE units (projections) between attention units.
                fillers: list of (generator, pulls_per_slot) consumed at
                the given rate to keep PE work evenly interleaved."""
                prev = None
                credit = [0.0] * len(fillers)
                slot = 0
                for h in range(HL):
                    oT = normp.tile([64, 1024], F32, tag="oT", name="oT")
                    zraw = normp.tile([1, 1024], F32, tag="zraw",
                                      name="zraw")
                    for u, (scores, exp_av, pso, c) in enumerate(
                            att_units(h, half)):
                        scores()
                        if prev is not None:
                            prev()
                        for fi, (fill, rate) in enumerate(fillers):
                            credit[fi] += rate
                            while credit[fi] >= 1.0:
                                credit[fi] -= 1.0
                                try:
                                    next(fill)
                                except StopIteration:
                                    credit[fi] = 0.0
                                    break
                        slot += 1
                        prev = exp_av
                        if u % 8 == 7:
                            # chunk done after its exp_av runs; stage it
                            cc = u // 8
                            def stage(pso=pso, cc=cc, oT=oT, zraw=zraw,
                                      ea=exp_av):
                                ea()
                                nc.vector.tensor_copy(
                                    out=oT[:, cc * 512:(cc + 1) * 512],
                                    in_=pso[0:64, :])
                                nc.vector.tensor_copy(
                                    out=zraw[:, cc * 512:(cc + 1) * 512],
                                    in_=pso[64:65, :])
                            prev = stage
                    # flush last unit of head then normalize
                    prev()
                    prev = None
                    normalize(h, half, oT, zraw)

            # ---------- schedule ----------
            # fillers consumed in order: pair0 KQ proj runs FIRST (ahead
            # of attention emission), then V-proj + remaining KQ pairs
            # interleave into half-0 attention, O-proj(half0) into half-1.
            for _ in kq_proj_units([0]):
                pass
            # V-proj must stay ahead of attention's av consumption in
            # head 0 (vt[2u+1] needed by slot u): 2 units/slot early.
            # KQ pairs 1-3 (24 units) spread so pair p completes well
            # before head 2p's attention starts (slot 32p).
            f_v = v_proj_units()
            f_kq = kq_proj_units([1, 2, 3])
            attention_half(0, [(f_v, 2.0), (f_kq, 0.34)])
            for f in (f_v, f_kq):
                for _ in f:
                    pass
            f_o = o_proj_units(range(NT // 2))
            attention_half(1, [(f_o, 0.17)])
            for _ in f_o:
                pass
            for _ in o_proj_units(range(NT // 2, NT)):
                pass

    nc.compile()
    return nc


_NC_CACHE = None


def _get_nc():
    global _NC_CACHE
    if _NC_CACHE is None:
        _NC_CACHE = build_nc()
    return _NC_CACHE


def make_in_maps(query, key_, value, mask, w_q, b_q, w_k, b_k, w_v, b_v,
                 w_o, b_o):
    import ml_dtypes
    f32 = np.float32
    bf16 = ml_dtypes.bfloat16
    c = lambda a: np.ascontiguousarray(a).astype(bf16)
    in_maps = []
    for core in range(N_CORES):
        b, g = core // 2, core % 2
        fs = slice(g * FL, (g + 1) * FL)
        # row-64 mask bias, pre-multiplied by 8 (exp applies scale 1/8)
        mrow = np.where(mask[b], 0.0, -240.0).astype(f32)
        in_maps.append({
            "qT": c(query[b].T.astype(f32, copy=False)),
            "kT": c(key_[b].T.astype(f32, copy=False)),
            "vT": c(value[b].T.astype(f32, copy=False)),
            "wqT": c(w_q[fs, :].T.astype(f32, copy=False)),
            "wkT": c(w_k[fs, :].T.astype(f32, copy=False)),
            "wvT": c(w_v[fs, :].T.astype(f32, copy=False)),
            "woT": c(w_o[:, fs].T.astype(f32, copy=False)),
            "bq": np.ascontiguousarray(
                b_q[fs].astype(f32, copy=False).reshape(NFT, 128).T),
            "bk": np.ascontiguousarray(
                b_k[fs].astype(f32, copy=False).reshape(NFT, 128).T),
            "bv": b_v[fs].reshape(1, FL).astype(bf16),
            "bo": (b_o.astype(f32, copy=False) if g == 0
                   else np.zeros(E, f32)).reshape(1, E).astype(bf16),
            "ones_d": np.ones((1, 128), bf16),
            "onesrow": np.ones((1, T), bf16),
            "maskrow": mrow.reshape(1, T).astype(bf16),
        })
    return in_maps


def kernel(query=None, key_=None, value=None, mask=None, w_q=None, b_q=None,
           w_k=None, b_k=None, w_v=None, b_v=None, w_o=None, b_o=None,
           key=None, **_kwargs):
    if key_ is None:
        key_ = key
    args = [np.asarray(a) for a in
            (query, key_, value, mask, w_q, b_q, w_k, b_k, w_v, b_v,
             w_o, b_o)]
    nc = _get_nc()
    in_maps = make_in_maps(*args)
    res = run_bass_kernel_spmd(nc, in_maps, core_ids=list(range(N_CORES)))
    outs = [res.results[i]["out"] for i in range(N_CORES)]
    full = np.empty((B, T, E), np.float32)
    for b in range(B):
        full[b] = outs[2 * b] + outs[2 * b + 1]
    return full
